# revision 11
# baseline (speedup 1.0000x reference)
"""Trainium2 Bass kernel for nn_GruAgent — optimized recurrence chain (v2).

Data-parallel over envs: 8 cores x 64 envs. Per core the GRU runs in
[h-dim on partitions, envs on free] layout. Per-step critical chain is
MM_rz -> sigmoid(r) -> p -> q -> tanh(n) -> t, with:
  - the (1-z)n + z*h state blend and reset-mask folded into MM_rz via
    stacked [U_rz | -U_rz] weights applied to [B; t] where B = z*m*mask'
    and t = n*(z-1)*mask' (so m' = B - t needs no extra hop before it)
  - b_hhn folded into the existing scalar_tensor_tensor that forms
    r*(gh_n + b_hhn)
  - gate activations write both partition halves via the scalar engine
    (the only engine allowed to cross partition bases) so every DVE op
    keeps same-base operands (hardware rule NCC_IBIR297)
  - bulk work (x transpose, gi GEMMs, masks, actor/critic head) is
    chunked and sprinkled between chain steps so the engine FIFOs never
    block the serial recurrence; recurrence/bulk matmuls run in bf16
    (4x the fp32 row rate), PSUM accumulation stays fp32
"""

import os
import sys

import numpy as np

for _p in ("/opt/trn_rl_repo", os.path.expanduser("~/.axon_site/_ro/trn_rl_repo")):
    if os.path.isdir(_p) and _p not in sys.path:
        sys.path.insert(0, _p)
        break

import concourse.bass as bass
import concourse.mybir as mybir
import concourse.tile as tile
from concourse import bacc
from concourse.masks import make_identity

T, B, OBS, H, A, L = 512, 512, 64, 64, 6, 64
N_CORES = 8
BL = B // N_CORES          # 64 envs per core
GS = 8                     # timesteps per group
COLS = GS * BL             # 512 columns per group
H3 = 3 * H

F32 = mybir.dt.float32
BF16 = mybir.dt.bfloat16
AF = mybir.ActivationFunctionType
ALU = mybir.AluOpType

WEIGHT_KEYS = [
    "w_ih", "w_hh", "b_ih", "b_hh",
    "aw1", "ab1", "aw2", "ab2", "aw3", "ab3",
    "cw1", "cb1", "cw2", "cb2", "cw3", "cb3",
]


def build(nc, t_loc=T):
    from contextlib import ExitStack

    assert t_loc % GS == 0
    ng = t_loc // GS

    x_d = nc.dram_tensor("x", [t_loc, BL, OBS], F32, kind="ExternalInput")
    done_d = nc.dram_tensor("done", [t_loc, BL], F32, kind="ExternalInput")
    h0_d = nc.dram_tensor("h0", [BL, H], F32, kind="ExternalInput")
    wih_d = nc.dram_tensor("w_ih", [H3, OBS], F32, kind="ExternalInput")
    whh_d = nc.dram_tensor("w_hh", [H3, H], F32, kind="ExternalInput")
    bih_d = nc.dram_tensor("b_ih", [H3], F32, kind="ExternalInput")
    bhh_d = nc.dram_tensor("b_hh", [H3], F32, kind="ExternalInput")
    aw1_d = nc.dram_tensor("aw1", [L, H + OBS], F32, kind="ExternalInput")
    ab1_d = nc.dram_tensor("ab1", [L], F32, kind="ExternalInput")
    aw2_d = nc.dram_tensor("aw2", [L, L], F32, kind="ExternalInput")
    ab2_d = nc.dram_tensor("ab2", [L], F32, kind="ExternalInput")
    aw3_d = nc.dram_tensor("aw3", [A, L], F32, kind="ExternalInput")
    ab3_d = nc.dram_tensor("ab3", [A], F32, kind="ExternalInput")
    cw1_d = nc.dram_tensor("cw1", [L, H + OBS], F32, kind="ExternalInput")
    cb1_d = nc.dram_tensor("cb1", [L], F32, kind="ExternalInput")
    cw2_d = nc.dram_tensor("cw2", [L, L], F32, kind="ExternalInput")
    cb2_d = nc.dram_tensor("cb2", [L], F32, kind="ExternalInput")
    cw3_d = nc.dram_tensor("cw3", [1, L], F32, kind="ExternalInput")
    cb3_d = nc.dram_tensor("cb3", [1], F32, kind="ExternalInput")
    out_d = nc.dram_tensor("out", [t_loc, BL, A + 1], F32, kind="ExternalOutput")

    with tile.TileContext(nc) as tc, ExitStack() as ctx:
        wp = ctx.enter_context(tc.tile_pool(name="wp", bufs=1))
        ldp = ctx.enter_context(tc.tile_pool(name="ldp", bufs=2))
        xtp = ctx.enter_context(tc.tile_pool(name="xtp", bufs=2))
        xnp = ctx.enter_context(tc.tile_pool(name="xnp", bufs=2))
        drp = ctx.enter_context(tc.tile_pool(name="drp", bufs=2))
        mbp = ctx.enter_context(tc.tile_pool(name="mbp", bufs=3))
        hzp = ctx.enter_context(tc.tile_pool(name="hzp", bufs=3))   # hszm group tiles
        small = ctx.enter_context(tc.tile_pool(name="small", bufs=4))
        qp = ctx.enter_context(tc.tile_pool(name="qp", bufs=3))
        tmlp = ctx.enter_context(tc.tile_pool(name="tmlp", bufs=2))
        onp = ctx.enter_context(tc.tile_pool(name="onp", bufs=2))

        przp = ctx.enter_context(tc.tile_pool(name="przp", bufs=2, space="PSUM"))
        pginp = ctx.enter_context(tc.tile_pool(name="pginp", bufs=2, space="PSUM"))
        pghnp = ctx.enter_context(tc.tile_pool(name="pghnp", bufs=2, space="PSUM"))
        pmisc = ctx.enter_context(tc.tile_pool(name="pmisc", bufs=2, space="PSUM"))

        ident = wp.tile([128, 128], F32, tag="ident")
        make_identity(nc, ident[:])

        def load_transposed(dram_ap, rows, cols, tag):
            """dram [rows, cols] -> sbuf tile [cols, rows]."""
            dst = wp.tile([cols, rows], F32, tag=tag)
            r0 = 0
            while r0 < rows:
                rr = min(128, rows - r0)
                tmp = ldp.tile([128, 128], F32, tag="wtmp")
                nc.sync.dma_start(tmp[:rr, :cols], dram_ap[r0:r0 + rr, :])
                pt = pmisc.tile([128, COLS], F32, tag="pm")
                nc.tensor.transpose(pt[:cols, :rr], tmp[:rr, :cols], ident[:rr, :rr])
                nc.scalar.copy(dst[:, r0:r0 + rr], pt[:cols, :rr])
                r0 += rr
            return dst

        def load_col(dram_1d, n, tag, off=0, dst=None, dst_off=0):
            if dst is None:
                dst = wp.tile([max(n + dst_off, 1), 1], F32, tag=tag)
            nc.sync.dma_start(
                dst[dst_off:dst_off + n, :],
                dram_1d[off:off + n].rearrange("p -> p ()"),
            )
            return dst

        # --- weights / constants preprocessing ---
        w_ihT = load_transposed(wih_d[:], H3, OBS, "wihT")    # [64, 192]
        w_hhT = load_transposed(whh_d[:], H3, H, "whhT")      # [64, 192]
        h0T = load_transposed(h0_d[:], BL, H, "h0T")          # [64, 64]

        # [U_rz | -U_rz] stacked along K: [128, 128] (bf16)
        rzUU = wp.tile([128, 128], BF16, tag="rzUU")
        nc.scalar.copy(rzUU[0:64, :], w_hhT[:, 0:128])
        nc.scalar.activation(rzUU[64:128, :], w_hhT[:, 0:128], AF.Identity,
                             scale=-1.0)
        nT_bf = wp.tile([64, 64], BF16, tag="nT_bf")
        nc.scalar.copy(nT_bf[:], w_hhT[:, 128:H3])
        # [U_n | -U_n] so the n-gate matmul reads [B; t] like the rz gate
        nUU = wp.tile([128, 64], BF16, tag="nUU")
        nc.scalar.copy(nUU[0:64, :], w_hhT[:, 128:H3])
        nc.scalar.activation(nUU[64:128, :], w_hhT[:, 128:H3], AF.Identity,
                             scale=-1.0)
        wih_bf = wp.tile([64, H3], BF16, tag="wih_bf")
        nc.scalar.copy(wih_bf[:], w_ihT[:])
        h0T_bf = wp.tile([64, 64], BF16, tag="h0T_bf")
        nc.scalar.copy(h0T_bf[:], h0T[:])

        # head layer1 weights: h-part [64, 128], x-part [64, 128]
        # (cols 0:64 actor, 64:128 critic)
        lhsT1h = wp.tile([64, 128], BF16, tag="lhsT1h")
        lhsT1x = wp.tile([64, 128], BF16, tag="lhsT1x")
        for src, c0 in ((aw1_d, 0), (cw1_d, 64)):
            tmp = ldp.tile([128, 128], F32, tag="wtmp")
            nc.sync.dma_start(tmp[:L, :H + OBS], src[:, :])
            pt = pmisc.tile([128, COLS], F32, tag="pm")
            nc.tensor.transpose(pt[:H, :L], tmp[:L, 0:H], ident[:L, :L])
            nc.tensor.transpose(pt[:OBS, 128:128 + L], tmp[:L, H:H + OBS],
                                ident[:L, :L])
            nc.scalar.copy(lhsT1h[:, c0:c0 + L], pt[:H, :L])
            nc.scalar.copy(lhsT1x[:, c0:c0 + L], pt[:OBS, 128:128 + L])

        lhsT2 = wp.tile([128, 128], BF16, tag="lhsT2")
        nc.vector.memset(lhsT2[:], 0.0)
        for src, o in ((aw2_d, 0), (cw2_d, 64)):
            tmp = ldp.tile([128, 128], F32, tag="wtmp")
            nc.sync.dma_start(tmp[:L, :L], src[:, :])
            pt = pmisc.tile([128, COLS], F32, tag="pm")
            nc.tensor.transpose(pt[:L, :L], tmp[:L, :L], ident[:L, :L])
            nc.scalar.copy(lhsT2[o:o + L, o:o + L], pt[:L, :L])

        lhsT3 = wp.tile([128, A + 1], BF16, tag="lhsT3")
        nc.vector.memset(lhsT3[:], 0.0)
        tmp = ldp.tile([128, 128], F32, tag="wtmp")
        nc.sync.dma_start(tmp[:A, :L], aw3_d[:, :])
        pt = pmisc.tile([128, COLS], F32, tag="pm")
        nc.tensor.transpose(pt[:L, :A], tmp[:A, :L], ident[:A, :A])
        nc.scalar.copy(lhsT3[:L, :A], pt[:L, :A])
        tmp = ldp.tile([128, 128], F32, tag="wtmp")
        nc.sync.dma_start(tmp[:1, :L], cw3_d[:, :])
        pt = pmisc.tile([128, COLS], F32, tag="pm")
        nc.tensor.transpose(pt[:L, :1], tmp[:1, :L], ident[:1, :1])
        nc.scalar.copy(lhsT3[64:64 + L, A:A + 1], pt[:L, :1])

        # biases
        bihc = load_col(bih_d, 128, "bihc")
        bhhc = load_col(bhh_d, 128, "bhhc")
        bias_r = wp.tile([64, 1], F32, tag="bias_r")
        nc.vector.tensor_add(bias_r[:], bihc[0:64, :], bhhc[0:64, :])
        bihz = load_col(bih_d, H, "bihz", off=64)
        bhhz = load_col(bhh_d, H, "bhhz", off=64)
        bias_z = wp.tile([64, 1], F32, tag="bias_z")
        nc.vector.tensor_add(bias_z[:], bihz[:], bhhz[:])
        b_ihn = load_col(bih_d, H, "b_ihn", off=128)          # [64,1]
        b_hhn = load_col(bhh_d, H, "b_hhn", off=128)          # [64,1]

        bias1 = wp.tile([128, 1], F32, tag="bias1")
        load_col(ab1_d, L, "bias1", dst=bias1, dst_off=0)
        load_col(cb1_d, L, "bias1", dst=bias1, dst_off=64)
        bias2 = wp.tile([128, 1], F32, tag="bias2")
        load_col(ab2_d, L, "bias2", dst=bias2, dst_off=0)
        load_col(cb2_d, L, "bias2", dst=bias2, dst_off=64)
        bias3 = wp.tile([A + 1, 1], F32, tag="bias3")
        load_col(ab3_d, A, "bias3", dst=bias3, dst_off=0)
        load_col(cb3_d, 1, "bias3", dst=bias3, dst_off=A)

        ones_row = wp.tile([1, 128], F32, tag="ones_row")
        nc.vector.memset(ones_row[:], 1.0)

        # state tiles (ping-pong): m at partitions 0:64
        mextA = wp.tile([64, BL], BF16, tag="mextA")
        mextB = wp.tile([64, BL], BF16, tag="mextB")

        def bulk_dma(g):
            """Issue the group's DMAs immediately; defer compute into ops."""
            xn = xnp.tile([128, GS // 2, OBS], F32, tag="xn")
            nc.sync.dma_start(
                xn[:],
                x_d[g * GS:(g + 1) * GS].rearrange("(k ph) b f -> (ph b) k f", ph=2),
            )
            dr = drp.tile([1, COLS], F32, tag="dr")
            nc.sync.dma_start(
                dr[:], done_d[g * GS:(g + 1) * GS].rearrange("t b -> () (t b)")
            )
            refs = dict(xn=xn, dr=dr)

            xT = xtp.tile([64, COLS], BF16, tag="xT")
            mb = mbp.tile([128, COLS], BF16, tag="mb")
            prz = przp.tile([128, COLS], F32, tag="prz")
            pgin = pginp.tile([64, COLS], F32, tag="pgin")
            refs.update(xT=xT, mb=mb, prz=prz, pgin=pgin)

            # mask ops run eagerly at the consuming chain's start (they must
            # precede the previous chain's last-step read of mb)
            mask_ops = []
            pmb = pmisc.tile([128, COLS], F32, tag="pm")
            for c in range(2):
                hc = bass.ts(c, COLS // 2)
                mask_ops.append(lambda hc=hc: nc.tensor.matmul(
                    pmb[:, hc], ones_row[:], dr[:, hc], start=True, stop=True,
                    skip_group_check=True))
                mask_ops.append(lambda hc=hc: nc.scalar.activation(
                    mb[:, hc], pmb[:, hc], AF.Identity, scale=-1.0, bias=1.0))
            refs["mask_ops"] = mask_ops

            ops = []
            ptx = pmisc.tile([128, COLS], F32, tag="pm")
            for k in range(GS // 2):
                ops.append(lambda k=k: nc.tensor.transpose(
                    ptx[:OBS, k * 128:(k + 1) * 128], xn[:, k, :], ident[:, :]
                ))
            for c in range(2):
                hc = bass.ts(c, COLS // 2)
                ops.append(lambda hc=hc: nc.vector.tensor_copy(
                    xT[:, hc], ptx[:OBS, hc]))
            # NOTE: start=True marks the whole 2KB PSUM partition-row
            # pending-zero, so only the FIRST chunk may use start=True;
            # later chunks write into already-pending elements (hw zeroes
            # on write) without invalidating earlier chunks.
            for c in range(2):
                hc = bass.ts(c, COLS // 2)
                ops.append(lambda hc=hc, c=c: nc.tensor.matmul(
                    prz[:, hc], wih_bf[:, 0:128], xT[:, hc],
                    start=(c == 0), stop=False, skip_group_check=True,
                ))
                ops.append(lambda hc=hc, c=c: nc.tensor.matmul(
                    pgin[:, hc], wih_bf[:, 128:H3], xT[:, hc],
                    start=(c == 0), stop=(c == 1), skip_group_check=True,
                ))
            return refs, ops

        state = {}

        def chain(g, refs, refs_next, sprinkle):
            prz, mb, pgin = refs["prz"], refs["mb"], refs["pgin"]
            hs = hzp.tile([64, COLS], BF16, tag="hs")
            refs["hs"] = hs
            for s in range(GS):
                t = g * GS + s
                cs = bass.ts(s, BL)
                last = t == t_loc - 1
                mext = state["mext"]

                # hidden-state matmuls (rz folded over [B; t], n over m)
                if t > 0:
                    bt = state["bt"]
                    nc.tensor.matmul(
                        prz[:, cs], rzUU[:], bt[:],
                        start=False, stop=(s == GS - 1), skip_group_check=True,
                    )
                else:
                    nc.tensor.matmul(
                        prz[:, cs], rzUU[0:64, :], mext[:],
                        start=False, stop=(s == GS - 1), skip_group_check=True,
                    )
                pghn = pghnp.tile([64, BL], F32, tag="pghn")
                if t > 0:
                    nc.tensor.matmul(
                        pghn[:], nUU[:], state["bt"][:], start=True, stop=True
                    )
                else:
                    nc.tensor.matmul(
                        pghn[:], nT_bf[:], mext[:], start=True, stop=True
                    )

                # gates (ACT is the only engine allowed to cross partition
                # bases, and only with its single tensor input)
                r_lo = small.tile([64, BL], F32, tag="r_lo")
                nc.scalar.activation(
                    r_lo[:], prz[0:64, cs], AF.Sigmoid, bias=bias_r[:]
                )
                z_hi = small.tile([128, BL], BF16, tag="z_hi")
                nc.scalar.activation(
                    z_hi[64:128, :], prz[64:128, cs], AF.Sigmoid, bias=bias_z[:]
                )
                # z/n duplicates at lo come from Pool single-input copies
                # (cross-base is legal for one-input ops; ACT stays lighter)
                z_lo = small.tile([64, BL], BF16, tag="z_lo")
                nc.gpsimd.tensor_copy(z_lo[:], z_hi[64:128, :])
                u_lo = small.tile([64, BL], BF16, tag="u_lo")
                nc.gpsimd.tensor_scalar(
                    u_lo[:], z_lo[:], -1.0, 1.0, ALU.mult, ALU.add,
                )
                # q = r * (U_n m + b_hhn) + gi_n
                p = qp.tile([64, BL], F32, tag="p")
                nc.vector.scalar_tensor_tensor(
                    p[:], pghn[:], b_hhn[:], r_lo[:], ALU.add, ALU.mult
                )
                q = qp.tile([64, BL], F32, tag="q")
                nc.vector.tensor_add(q[:], p[:], pgin[:, cs])
                # n at hi (on-path), lo duplicate via Pool copy
                n_hi = small.tile([128, BL], BF16, tag="n_hi")
                nc.scalar.activation(
                    n_hi[64:128, :], q[:], AF.Tanh, bias=b_ihn[:],
                )
                n_lo = small.tile([64, BL], BF16, tag="n_lo")
                nc.gpsimd.tensor_copy(n_lo[:], n_hi[64:128, :])
                # zm = z*m (head + B)
                zm = small.tile([64, BL], BF16, tag="zm")
                nc.vector.tensor_mul(zm[:], z_lo[:], mext[:])

                if not last:
                    if s == GS - 1:
                        mbn = refs_next["mb"]
                        csn = bass.ts(0, BL)
                    else:
                        mbn = mb
                        csn = bass.ts(s + 1, BL)
                    bt = small.tile([128, BL], BF16, tag="bt")
                    # B = zm*mask' at lo -> bt[0:]
                    nc.vector.tensor_mul(bt[0:64, :], zm[:], mbn[0:64, csn])
                    # A = (z-1)*mask' at hi; t = n*A -> bt[64:] (on-path)
                    a_hi = small.tile([128, BL], BF16, tag="a_hi")
                    nc.vector.scalar_tensor_tensor(
                        a_hi[64:128, :], z_hi[64:128, :], -1.0, mbn[64:128, csn],
                        ALU.add, ALU.mult,
                    )
                    nc.vector.tensor_mul(bt[64:128, :], n_hi[64:128, :],
                                         a_hi[64:128, :])
                    # off-path: t at lo, m' = B - t_lo (feeds zm next step)
                    a_lo = small.tile([64, BL], BF16, tag="a_lo")
                    nc.vector.scalar_tensor_tensor(
                        a_lo[:], z_lo[:], -1.0, mbn[0:64, csn],
                        ALU.add, ALU.mult,
                    )
                    t_lo = small.tile([64, BL], BF16, tag="t_lo")
                    nc.vector.tensor_mul(t_lo[:], n_lo[:], a_lo[:])
                    mext2 = mextA if ((t + 1) % 2 == 0) else mextB
                    nc.vector.tensor_sub(mext2[:], bt[0:64, :], t_lo[:])
                    state["bt"] = bt
                    state["mext"] = mext2
                # hs for the head (fully off the recurrence path)
                t2 = small.tile([64, BL], BF16, tag="t2")
                nc.gpsimd.tensor_mul(t2[:], n_lo[:], u_lo[:])
                nc.gpsimd.tensor_add(hs[:, cs], t2[:], zm[:])
                # emit a slice of the background queue after each step
                rem = GS - s
                k = (len(sprinkle) + rem - 1) // rem
                for _ in range(min(k, len(sprinkle))):
                    sprinkle.pop(0)()
            while sprinkle:
                sprinkle.pop(0)()

        def head_ops(g, refs):
            hszm, xT = refs["hs"], refs["xT"]
            p1 = pmisc.tile([128, COLS], F32, tag="pm")
            t1 = tmlp.tile([128, COLS], BF16, tag="t1")
            p2 = pmisc.tile([128, COLS], F32, tag="pm")
            t2 = tmlp.tile([128, COLS], BF16, tag="t2")
            p3 = pmisc.tile([128, COLS], F32, tag="pm")
            o7 = tmlp.tile([A + 1, COLS], F32, tag="o7")
            ops = []
            for c in range(2):
                hc = bass.ts(c, COLS // 2)
                ops.append(lambda hc=hc: nc.tensor.matmul(
                    p1[:, hc], lhsT1h[:], hszm[:, hc], start=True, stop=False,
                    skip_group_check=True))
                ops.append(lambda hc=hc: nc.tensor.matmul(
                    p1[:, hc], lhsT1x[:], xT[:, hc], start=False, stop=True,
                    skip_group_check=True))
                ops.append(lambda hc=hc: nc.scalar.activation(
                    t1[:, hc], p1[:, hc], AF.Tanh, bias=bias1[:]))
                ops.append(lambda hc=hc: nc.tensor.matmul(
                    p2[:, hc], lhsT2[:], t1[:, hc], start=True, stop=True,
                    skip_group_check=True))
                ops.append(lambda hc=hc: nc.scalar.activation(
                    t2[:, hc], p2[:, hc], AF.Tanh, bias=bias2[:]))
                ops.append(lambda hc=hc: nc.tensor.matmul(
                    p3[:A + 1, hc], lhsT3[:], t2[:, hc], start=True, stop=True,
                    skip_group_check=True))
                ops.append(lambda hc=hc: nc.scalar.activation(
                    o7[:, hc], p3[:A + 1, hc], AF.Identity, bias=bias3[:]))
            po = pmisc.tile([128, GS // 2, A + 1], F32, tag="pm")
            for k in range(GS // 2):
                ops.append(lambda k=k: nc.tensor.transpose(
                    po[:, k, :], o7[:, k * 128:(k + 1) * 128],
                    ident[:A + 1, :A + 1]))
            on = onp.tile([128, GS // 2, A + 1], F32, tag="on")
            for c in range(2):
                ops.append(lambda c=c: nc.vector.tensor_copy(
                    on[:, c * 2:(c + 1) * 2, :], po[:, c * 2:(c + 1) * 2, :]))
            ops.append(lambda: nc.sync.dma_start(
                out_d[g * GS:(g + 1) * GS].rearrange(
                    "(k ph) b j -> (ph b) k j", ph=2),
                on[:],
            ))
            return ops

        all_refs = {}
        all_refs[0], ops0 = bulk_dma(0)
        for op in all_refs[0]["mask_ops"] + ops0:
            op()
        # m_0 = mask_0 * h0  into mextA
        nc.vector.tensor_mul(mextA[:], h0T_bf[:], all_refs[0]["mb"][0:64, 0:BL])
        state["mext"] = mextA
        prev_head = []
        for g in range(ng):
            if g + 1 < ng:
                all_refs[g + 1], bops = bulk_dma(g + 1)
                # next group's mask must exist before this chain's last step
                for op in all_refs[g + 1]["mask_ops"]:
                    op()
            else:
                bops = []
            # head ops first: they reuse pmisc buffers that the later bulk
            # ops of the following group will overwrite (emission order is
            # program order, so readers must be emitted before new writers)
            chain(g, all_refs[g], all_refs.get(g + 1), prev_head + bops)
            prev_head = head_ops(g, all_refs[g])
            all_refs.pop(g - 1, None)
        for op in prev_head:
            op()

    return nc


_BUILT = {}


def get_built(t_loc=T):
    if t_loc not in _BUILT:
        nc = bacc.Bacc(None, target_bir_lowering=False)
        build(nc, t_loc)
        nc.compile()
        _BUILT[t_loc] = nc
    return _BUILT[t_loc]


def shard_inputs(inputs, t_loc=T):
    x = np.ascontiguousarray(np.asarray(inputs["x"], np.float32)).reshape(t_loc, B, OBS)
    done = np.ascontiguousarray(np.asarray(inputs["done"], np.float32)).reshape(t_loc, B)
    h0 = np.ascontiguousarray(np.asarray(inputs["gru_state"], np.float32)).reshape(B, H)
    common = {
        k: np.ascontiguousarray(np.asarray(inputs[k], np.float32))
        for k in WEIGHT_KEYS
    }
    in_maps = []
    for c in range(N_CORES):
        sl = slice(c * BL, (c + 1) * BL)
        m = dict(common)
        m["x"] = np.ascontiguousarray(x[:, sl, :])
        m["done"] = np.ascontiguousarray(done[:, sl])
        m["h0"] = np.ascontiguousarray(h0[sl, :])
        in_maps.append(m)
    return in_maps


def assemble_output(per_core_outs, t_loc=T):
    outs = [np.asarray(o, np.float32).reshape(t_loc, BL, A + 1) for o in per_core_outs]
    full = np.stack(outs, axis=1).reshape(t_loc, B, A + 1)
    return np.ascontiguousarray(full.reshape(t_loc * B, A + 1))


def run_on_hw(inputs, t_loc=T, trace=False, **kw):
    from concourse.bass_utils import run_bass_kernel_spmd

    nc = get_built(t_loc)
    in_maps = shard_inputs(inputs, t_loc)
    res = run_bass_kernel_spmd(
        nc, in_maps, core_ids=list(range(N_CORES)), trace=trace, **kw
    )
    out = assemble_output([r["out"] for r in res.results], t_loc)
    return out, res


def kernel(**inputs):
    out, _ = run_on_hw(inputs)
    return out


# revision 12
# speedup vs baseline: 1.0785x; 1.0785x over previous
"""Trainium2 Bass kernel for nn_GruAgent — optimized recurrence chain (v2).

Data-parallel over envs: 8 cores x 64 envs. Per core the GRU runs in
[h-dim on partitions, envs on free] layout. Per-step critical chain is
MM_rz -> sigmoid(r) -> p -> q -> tanh(n) -> t, with:
  - the (1-z)n + z*h state blend and reset-mask folded into MM_rz via
    stacked [U_rz | -U_rz] weights applied to [B; t] where B = z*m*mask'
    and t = n*(z-1)*mask' (so m' = B - t needs no extra hop before it)
  - b_hhn folded into the existing scalar_tensor_tensor that forms
    r*(gh_n + b_hhn)
  - gate activations write both partition halves via the scalar engine
    (the only engine allowed to cross partition bases) so every DVE op
    keeps same-base operands (hardware rule NCC_IBIR297)
  - bulk work (x transpose, gi GEMMs, masks, actor/critic head) is
    chunked and sprinkled between chain steps so the engine FIFOs never
    block the serial recurrence; recurrence/bulk matmuls run in bf16
    (4x the fp32 row rate), PSUM accumulation stays fp32
"""

import os
import sys

import numpy as np

for _p in ("/opt/trn_rl_repo", os.path.expanduser("~/.axon_site/_ro/trn_rl_repo")):
    if os.path.isdir(_p) and _p not in sys.path:
        sys.path.insert(0, _p)
        break

import concourse.bass as bass
import concourse.mybir as mybir
import concourse.tile as tile
from concourse import bacc
from concourse.masks import make_identity

T, B, OBS, H, A, L = 512, 512, 64, 64, 6, 64
N_CORES = 8
BL = B // N_CORES          # 64 envs per core
GS = 8                     # timesteps per group
COLS = GS * BL             # 512 columns per group
H3 = 3 * H

F32 = mybir.dt.float32
BF16 = mybir.dt.bfloat16
AF = mybir.ActivationFunctionType
ALU = mybir.AluOpType

WEIGHT_KEYS = [
    "w_ih", "w_hh", "b_ih", "b_hh",
    "aw1", "ab1", "aw2", "ab2", "aw3", "ab3",
    "cw1", "cb1", "cw2", "cb2", "cw3", "cb3",
]


def build(nc, t_loc=T):
    from contextlib import ExitStack

    assert t_loc % GS == 0
    ng = t_loc // GS

    x_d = nc.dram_tensor("x", [t_loc, BL, OBS], F32, kind="ExternalInput")
    done_d = nc.dram_tensor("done", [t_loc, BL], F32, kind="ExternalInput")
    h0_d = nc.dram_tensor("h0", [BL, H], F32, kind="ExternalInput")
    wih_d = nc.dram_tensor("w_ih", [H3, OBS], F32, kind="ExternalInput")
    whh_d = nc.dram_tensor("w_hh", [H3, H], F32, kind="ExternalInput")
    bih_d = nc.dram_tensor("b_ih", [H3], F32, kind="ExternalInput")
    bhh_d = nc.dram_tensor("b_hh", [H3], F32, kind="ExternalInput")
    aw1_d = nc.dram_tensor("aw1", [L, H + OBS], F32, kind="ExternalInput")
    ab1_d = nc.dram_tensor("ab1", [L], F32, kind="ExternalInput")
    aw2_d = nc.dram_tensor("aw2", [L, L], F32, kind="ExternalInput")
    ab2_d = nc.dram_tensor("ab2", [L], F32, kind="ExternalInput")
    aw3_d = nc.dram_tensor("aw3", [A, L], F32, kind="ExternalInput")
    ab3_d = nc.dram_tensor("ab3", [A], F32, kind="ExternalInput")
    cw1_d = nc.dram_tensor("cw1", [L, H + OBS], F32, kind="ExternalInput")
    cb1_d = nc.dram_tensor("cb1", [L], F32, kind="ExternalInput")
    cw2_d = nc.dram_tensor("cw2", [L, L], F32, kind="ExternalInput")
    cb2_d = nc.dram_tensor("cb2", [L], F32, kind="ExternalInput")
    cw3_d = nc.dram_tensor("cw3", [1, L], F32, kind="ExternalInput")
    cb3_d = nc.dram_tensor("cb3", [1], F32, kind="ExternalInput")
    out_d = nc.dram_tensor("out", [t_loc, BL, A + 1], F32, kind="ExternalOutput")

    with tile.TileContext(nc) as tc, ExitStack() as ctx:
        wp = ctx.enter_context(tc.tile_pool(name="wp", bufs=1))
        ldp = ctx.enter_context(tc.tile_pool(name="ldp", bufs=2))
        xtp = ctx.enter_context(tc.tile_pool(name="xtp", bufs=2))
        pgsp = ctx.enter_context(tc.tile_pool(name="pgsp", bufs=2))
        xnp = ctx.enter_context(tc.tile_pool(name="xnp", bufs=2))
        drp = ctx.enter_context(tc.tile_pool(name="drp", bufs=2))
        mbp = ctx.enter_context(tc.tile_pool(name="mbp", bufs=3))
        hzp = ctx.enter_context(tc.tile_pool(name="hzp", bufs=3))   # hszm group tiles
        small = ctx.enter_context(tc.tile_pool(name="small", bufs=4))
        qp = ctx.enter_context(tc.tile_pool(name="qp", bufs=3))
        tmlp = ctx.enter_context(tc.tile_pool(name="tmlp", bufs=2))
        onp = ctx.enter_context(tc.tile_pool(name="onp", bufs=2))

        przp = ctx.enter_context(tc.tile_pool(name="przp", bufs=2, space="PSUM"))
        pginp = ctx.enter_context(tc.tile_pool(name="pginp", bufs=2, space="PSUM"))
        pghnp = ctx.enter_context(tc.tile_pool(name="pghnp", bufs=2, space="PSUM"))
        pmisc = ctx.enter_context(tc.tile_pool(name="pmisc", bufs=2, space="PSUM"))

        ident = wp.tile([128, 128], F32, tag="ident")
        make_identity(nc, ident[:])

        def load_transposed(dram_ap, rows, cols, tag):
            """dram [rows, cols] -> sbuf tile [cols, rows]."""
            dst = wp.tile([cols, rows], F32, tag=tag)
            r0 = 0
            while r0 < rows:
                rr = min(128, rows - r0)
                tmp = ldp.tile([128, 128], F32, tag="wtmp")
                nc.sync.dma_start(tmp[:rr, :cols], dram_ap[r0:r0 + rr, :])
                pt = pmisc.tile([128, COLS], F32, tag="pm")
                nc.tensor.transpose(pt[:cols, :rr], tmp[:rr, :cols], ident[:rr, :rr])
                nc.scalar.copy(dst[:, r0:r0 + rr], pt[:cols, :rr])
                r0 += rr
            return dst

        def load_col(dram_1d, n, tag, off=0, dst=None, dst_off=0):
            if dst is None:
                dst = wp.tile([max(n + dst_off, 1), 1], F32, tag=tag)
            nc.sync.dma_start(
                dst[dst_off:dst_off + n, :],
                dram_1d[off:off + n].rearrange("p -> p ()"),
            )
            return dst

        # --- weights / constants preprocessing ---
        w_ihT = load_transposed(wih_d[:], H3, OBS, "wihT")    # [64, 192]
        w_hhT = load_transposed(whh_d[:], H3, H, "whhT")      # [64, 192]
        h0T = load_transposed(h0_d[:], BL, H, "h0T")          # [64, 64]

        # [U_rz | -U_rz] stacked along K: [128, 128] (bf16)
        rzUU = wp.tile([128, 128], BF16, tag="rzUU")
        nc.scalar.copy(rzUU[0:64, :], w_hhT[:, 0:128])
        nc.scalar.activation(rzUU[64:128, :], w_hhT[:, 0:128], AF.Identity,
                             scale=-1.0)
        nT_bf = wp.tile([64, 64], BF16, tag="nT_bf")
        nc.scalar.copy(nT_bf[:], w_hhT[:, 128:H3])
        # [U_n | -U_n] so the n-gate matmul reads [B; t] like the rz gate
        nUU = wp.tile([128, 64], BF16, tag="nUU")
        nc.scalar.copy(nUU[0:64, :], w_hhT[:, 128:H3])
        nc.scalar.activation(nUU[64:128, :], w_hhT[:, 128:H3], AF.Identity,
                             scale=-1.0)
        wih_bf = wp.tile([64, H3], BF16, tag="wih_bf")
        nc.scalar.copy(wih_bf[:], w_ihT[:])
        h0T_bf = wp.tile([64, 64], BF16, tag="h0T_bf")
        nc.scalar.copy(h0T_bf[:], h0T[:])

        # head layer1 weights: h-part [64, 128], x-part [64, 128]
        # (cols 0:64 actor, 64:128 critic)
        lhsT1h = wp.tile([64, 128], BF16, tag="lhsT1h")
        lhsT1x = wp.tile([64, 128], BF16, tag="lhsT1x")
        for src, c0 in ((aw1_d, 0), (cw1_d, 64)):
            tmp = ldp.tile([128, 128], F32, tag="wtmp")
            nc.sync.dma_start(tmp[:L, :H + OBS], src[:, :])
            pt = pmisc.tile([128, COLS], F32, tag="pm")
            nc.tensor.transpose(pt[:H, :L], tmp[:L, 0:H], ident[:L, :L])
            nc.tensor.transpose(pt[:OBS, 128:128 + L], tmp[:L, H:H + OBS],
                                ident[:L, :L])
            nc.scalar.copy(lhsT1h[:, c0:c0 + L], pt[:H, :L])
            nc.scalar.copy(lhsT1x[:, c0:c0 + L], pt[:OBS, 128:128 + L])

        lhsT2 = wp.tile([128, 128], BF16, tag="lhsT2")
        nc.vector.memset(lhsT2[:], 0.0)
        for src, o in ((aw2_d, 0), (cw2_d, 64)):
            tmp = ldp.tile([128, 128], F32, tag="wtmp")
            nc.sync.dma_start(tmp[:L, :L], src[:, :])
            pt = pmisc.tile([128, COLS], F32, tag="pm")
            nc.tensor.transpose(pt[:L, :L], tmp[:L, :L], ident[:L, :L])
            nc.scalar.copy(lhsT2[o:o + L, o:o + L], pt[:L, :L])

        lhsT3 = wp.tile([128, A + 1], BF16, tag="lhsT3")
        nc.vector.memset(lhsT3[:], 0.0)
        tmp = ldp.tile([128, 128], F32, tag="wtmp")
        nc.sync.dma_start(tmp[:A, :L], aw3_d[:, :])
        pt = pmisc.tile([128, COLS], F32, tag="pm")
        nc.tensor.transpose(pt[:L, :A], tmp[:A, :L], ident[:A, :A])
        nc.scalar.copy(lhsT3[:L, :A], pt[:L, :A])
        tmp = ldp.tile([128, 128], F32, tag="wtmp")
        nc.sync.dma_start(tmp[:1, :L], cw3_d[:, :])
        pt = pmisc.tile([128, COLS], F32, tag="pm")
        nc.tensor.transpose(pt[:L, :1], tmp[:1, :L], ident[:1, :1])
        nc.scalar.copy(lhsT3[64:64 + L, A:A + 1], pt[:L, :1])

        # biases
        bihc = load_col(bih_d, 128, "bihc")
        bhhc = load_col(bhh_d, 128, "bhhc")
        bias_r = wp.tile([64, 1], F32, tag="bias_r")
        nc.vector.tensor_add(bias_r[:], bihc[0:64, :], bhhc[0:64, :])
        bihz = load_col(bih_d, H, "bihz", off=64)
        bhhz = load_col(bhh_d, H, "bhhz", off=64)
        bias_z = wp.tile([64, 1], F32, tag="bias_z")
        nc.vector.tensor_add(bias_z[:], bihz[:], bhhz[:])
        b_ihn = load_col(bih_d, H, "b_ihn", off=128)          # [64,1]
        b_hhn = load_col(bhh_d, H, "b_hhn", off=128)          # [64,1]

        bias1 = wp.tile([128, 1], F32, tag="bias1")
        load_col(ab1_d, L, "bias1", dst=bias1, dst_off=0)
        load_col(cb1_d, L, "bias1", dst=bias1, dst_off=64)
        bias2 = wp.tile([128, 1], F32, tag="bias2")
        load_col(ab2_d, L, "bias2", dst=bias2, dst_off=0)
        load_col(cb2_d, L, "bias2", dst=bias2, dst_off=64)
        bias3 = wp.tile([A + 1, 1], F32, tag="bias3")
        load_col(ab3_d, A, "bias3", dst=bias3, dst_off=0)
        load_col(cb3_d, 1, "bias3", dst=bias3, dst_off=A)

        ones_row = wp.tile([1, 128], F32, tag="ones_row")
        nc.vector.memset(ones_row[:], 1.0)

        # state tiles (ping-pong): m at partitions 0:64
        mextA = wp.tile([64, BL], BF16, tag="mextA")
        mextB = wp.tile([64, BL], BF16, tag="mextB")

        def bulk_dma(g):
            """Issue the group's DMAs immediately; defer compute into ops."""
            xn = xnp.tile([128, GS // 2, OBS], F32, tag="xn")
            nc.sync.dma_start(
                xn[:],
                x_d[g * GS:(g + 1) * GS].rearrange("(k ph) b f -> (ph b) k f", ph=2),
            )
            dr = drp.tile([1, COLS], F32, tag="dr")
            nc.sync.dma_start(
                dr[:], done_d[g * GS:(g + 1) * GS].rearrange("t b -> () (t b)")
            )
            refs = dict(xn=xn, dr=dr)

            xT = xtp.tile([64, COLS], BF16, tag="xT")
            mb = mbp.tile([128, COLS], BF16, tag="mb")
            prz = przp.tile([128, COLS], F32, tag="prz")
            pgin = pginp.tile([64, COLS], F32, tag="pgin")
            pgin_sb = pgsp.tile([64, COLS], BF16, tag="pgin_sb")
            refs.update(xT=xT, mb=mb, prz=prz, pgin=pgin, pgin_sb=pgin_sb)

            # mask ops run eagerly at the consuming chain's start (they must
            # precede the previous chain's last-step read of mb)
            mask_ops = []
            pmb = pmisc.tile([128, COLS], F32, tag="pm")
            for c in range(2):
                hc = bass.ts(c, COLS // 2)
                mask_ops.append(lambda hc=hc: nc.tensor.matmul(
                    pmb[:, hc], ones_row[:], dr[:, hc], start=True, stop=True,
                    skip_group_check=True))
                mask_ops.append(lambda hc=hc: nc.scalar.activation(
                    mb[:, hc], pmb[:, hc], AF.Identity, scale=-1.0, bias=1.0))
            refs["mask_ops"] = mask_ops

            ops = []
            ptx = pmisc.tile([128, COLS], F32, tag="pm")
            for k in range(GS // 2):
                ops.append(lambda k=k: nc.tensor.transpose(
                    ptx[:OBS, k * 128:(k + 1) * 128], xn[:, k, :], ident[:, :]
                ))
            for c in range(2):
                hc = bass.ts(c, COLS // 2)
                ops.append(lambda hc=hc: nc.vector.tensor_copy(
                    xT[:, hc], ptx[:OBS, hc]))
            # NOTE: start=True marks the whole 2KB PSUM partition-row
            # pending-zero, so only the FIRST chunk may use start=True;
            # later chunks write into already-pending elements (hw zeroes
            # on write) without invalidating earlier chunks.
            for c in range(2):
                hc = bass.ts(c, COLS // 2)
                ops.append(lambda hc=hc, c=c: nc.tensor.matmul(
                    prz[:, hc], wih_bf[:, 0:128], xT[:, hc],
                    start=(c == 0), stop=False, skip_group_check=True,
                ))
                ops.append(lambda hc=hc, c=c: nc.tensor.matmul(
                    pgin[:, hc], wih_bf[:, 128:H3], xT[:, hc],
                    start=(c == 0), stop=(c == 1), skip_group_check=True,
                ))
                ops.append(lambda hc=hc: nc.vector.tensor_copy(
                    pgin_sb[:, hc], pgin[:, hc]))
            return refs, ops

        state = {}

        def chain(g, refs, refs_next, sprinkle):
            prz, mb = refs["prz"], refs["mb"]
            pgin_sb = refs["pgin_sb"]
            hs = hzp.tile([64, COLS], BF16, tag="hs")
            refs["hs"] = hs
            for s in range(GS):
                t = g * GS + s
                cs = bass.ts(s, BL)
                last = t == t_loc - 1
                mext = state["mext"]

                # hidden-state matmuls (rz folded over [B; t], n over m)
                if t > 0:
                    bt = state["bt"]
                    nc.tensor.matmul(
                        prz[:, cs], rzUU[:], bt[:],
                        start=False, stop=(s == GS - 1), skip_group_check=True,
                    )
                else:
                    nc.tensor.matmul(
                        prz[:, cs], rzUU[0:64, :], mext[:],
                        start=False, stop=(s == GS - 1), skip_group_check=True,
                    )
                pghn = pghnp.tile([64, BL], F32, tag="pghn")
                if t > 0:
                    nc.tensor.matmul(
                        pghn[:], nUU[:], state["bt"][:], start=True, stop=True
                    )
                else:
                    nc.tensor.matmul(
                        pghn[:], nT_bf[:], mext[:], start=True, stop=True
                    )

                # gates (ACT is the only engine allowed to cross partition
                # bases, and only with its single tensor input)
                r_lo = small.tile([64, BL], F32, tag="r_lo")
                nc.scalar.activation(
                    r_lo[:], prz[0:64, cs], AF.Sigmoid, bias=bias_r[:]
                )
                z_hi = small.tile([128, BL], BF16, tag="z_hi")
                nc.scalar.activation(
                    z_hi[64:128, :], prz[64:128, cs], AF.Sigmoid, bias=bias_z[:]
                )
                # z/n duplicates at lo come from Pool single-input copies
                # (cross-base is legal for one-input ops; ACT stays lighter)
                z_lo = small.tile([64, BL], BF16, tag="z_lo")
                nc.gpsimd.tensor_copy(z_lo[:], z_hi[64:128, :])
                u_lo = small.tile([64, BL], BF16, tag="u_lo")
                nc.gpsimd.tensor_scalar(
                    u_lo[:], z_lo[:], -1.0, 1.0, ALU.mult, ALU.add,
                )
                # q = r * (U_n m + b_hhn) + gi_n
                p = qp.tile([64, BL], BF16, tag="p")
                nc.vector.scalar_tensor_tensor(
                    p[:], pghn[:], b_hhn[:], r_lo[:], ALU.add, ALU.mult
                )
                q = qp.tile([64, BL], BF16, tag="q")
                nc.vector.tensor_add(q[:], p[:], pgin_sb[:, cs])
                # n at hi (on-path), lo duplicate via Pool copy
                n_hi = small.tile([128, BL], BF16, tag="n_hi")
                nc.scalar.activation(
                    n_hi[64:128, :], q[:], AF.Tanh, bias=b_ihn[:],
                )
                n_lo = small.tile([64, BL], BF16, tag="n_lo")
                nc.gpsimd.tensor_copy(n_lo[:], n_hi[64:128, :])
                # zm = z*m (head + B)
                zm = small.tile([64, BL], BF16, tag="zm")
                nc.vector.tensor_mul(zm[:], z_lo[:], mext[:])

                if not last:
                    if s == GS - 1:
                        mbn = refs_next["mb"]
                        csn = bass.ts(0, BL)
                    else:
                        mbn = mb
                        csn = bass.ts(s + 1, BL)
                    bt = small.tile([128, BL], BF16, tag="bt")
                    # B = zm*mask' at lo -> bt[0:]
                    nc.vector.tensor_mul(bt[0:64, :], zm[:], mbn[0:64, csn])
                    # A = (z-1)*mask' at hi; t = n*A -> bt[64:] (on-path)
                    a_hi = small.tile([128, BL], BF16, tag="a_hi")
                    nc.vector.scalar_tensor_tensor(
                        a_hi[64:128, :], z_hi[64:128, :], -1.0, mbn[64:128, csn],
                        ALU.add, ALU.mult,
                    )
                    nc.vector.tensor_mul(bt[64:128, :], n_hi[64:128, :],
                                         a_hi[64:128, :])
                    # off-path: t at lo, m' = B - t_lo (feeds zm next step)
                    a_lo = small.tile([64, BL], BF16, tag="a_lo")
                    nc.vector.scalar_tensor_tensor(
                        a_lo[:], z_lo[:], -1.0, mbn[0:64, csn],
                        ALU.add, ALU.mult,
                    )
                    t_lo = small.tile([64, BL], BF16, tag="t_lo")
                    nc.vector.tensor_mul(t_lo[:], n_lo[:], a_lo[:])
                    mext2 = mextA if ((t + 1) % 2 == 0) else mextB
                    nc.vector.tensor_sub(mext2[:], bt[0:64, :], t_lo[:])
                    state["bt"] = bt
                    state["mext"] = mext2
                # hs for the head (fully off the recurrence path)
                t2 = small.tile([64, BL], BF16, tag="t2")
                nc.gpsimd.tensor_mul(t2[:], n_lo[:], u_lo[:])
                nc.gpsimd.tensor_add(hs[:, cs], t2[:], zm[:])
                # emit a slice of the background queue after each step
                rem = GS - s
                k = (len(sprinkle) + rem - 1) // rem
                for _ in range(min(k, len(sprinkle))):
                    sprinkle.pop(0)()
            while sprinkle:
                sprinkle.pop(0)()

        def head_ops(g, refs):
            hszm, xT = refs["hs"], refs["xT"]
            p1 = pmisc.tile([128, COLS], F32, tag="pm")
            t1 = tmlp.tile([128, COLS], BF16, tag="t1")
            p2 = pmisc.tile([128, COLS], F32, tag="pm")
            t2 = tmlp.tile([128, COLS], BF16, tag="t2")
            p3 = pmisc.tile([128, COLS], F32, tag="pm")
            o7 = tmlp.tile([A + 1, COLS], F32, tag="o7")
            ops = []
            for c in range(2):
                hc = bass.ts(c, COLS // 2)
                ops.append(lambda hc=hc: nc.tensor.matmul(
                    p1[:, hc], lhsT1h[:], hszm[:, hc], start=True, stop=False,
                    skip_group_check=True))
                ops.append(lambda hc=hc: nc.tensor.matmul(
                    p1[:, hc], lhsT1x[:], xT[:, hc], start=False, stop=True,
                    skip_group_check=True))
                ops.append(lambda hc=hc: nc.scalar.activation(
                    t1[:, hc], p1[:, hc], AF.Tanh, bias=bias1[:]))
                ops.append(lambda hc=hc: nc.tensor.matmul(
                    p2[:, hc], lhsT2[:], t1[:, hc], start=True, stop=True,
                    skip_group_check=True))
                ops.append(lambda hc=hc: nc.scalar.activation(
                    t2[:, hc], p2[:, hc], AF.Tanh, bias=bias2[:]))
                ops.append(lambda hc=hc: nc.tensor.matmul(
                    p3[:A + 1, hc], lhsT3[:], t2[:, hc], start=True, stop=True,
                    skip_group_check=True))
                ops.append(lambda hc=hc: nc.scalar.activation(
                    o7[:, hc], p3[:A + 1, hc], AF.Identity, bias=bias3[:]))
            po = pmisc.tile([128, GS // 2, A + 1], F32, tag="pm")
            for k in range(GS // 2):
                ops.append(lambda k=k: nc.tensor.transpose(
                    po[:, k, :], o7[:, k * 128:(k + 1) * 128],
                    ident[:A + 1, :A + 1]))
            on = onp.tile([128, GS // 2, A + 1], F32, tag="on")
            for c in range(2):
                ops.append(lambda c=c: nc.vector.tensor_copy(
                    on[:, c * 2:(c + 1) * 2, :], po[:, c * 2:(c + 1) * 2, :]))
            ops.append(lambda: nc.sync.dma_start(
                out_d[g * GS:(g + 1) * GS].rearrange(
                    "(k ph) b j -> (ph b) k j", ph=2),
                on[:],
            ))
            return ops

        all_refs = {}
        all_refs[0], ops0 = bulk_dma(0)
        for op in all_refs[0]["mask_ops"] + ops0:
            op()
        # m_0 = mask_0 * h0  into mextA
        nc.vector.tensor_mul(mextA[:], h0T_bf[:], all_refs[0]["mb"][0:64, 0:BL])
        state["mext"] = mextA
        prev_head = []
        for g in range(ng):
            if g + 1 < ng:
                all_refs[g + 1], bops = bulk_dma(g + 1)
                # next group's mask must exist before this chain's last step
                for op in all_refs[g + 1]["mask_ops"]:
                    op()
            else:
                bops = []
            # head ops first: they reuse pmisc buffers that the later bulk
            # ops of the following group will overwrite (emission order is
            # program order, so readers must be emitted before new writers)
            chain(g, all_refs[g], all_refs.get(g + 1), prev_head + bops)
            prev_head = head_ops(g, all_refs[g])
            all_refs.pop(g - 1, None)
        for op in prev_head:
            op()

    return nc


_BUILT = {}


def get_built(t_loc=T):
    if t_loc not in _BUILT:
        nc = bacc.Bacc(None, target_bir_lowering=False)
        build(nc, t_loc)
        nc.compile()
        _BUILT[t_loc] = nc
    return _BUILT[t_loc]


def shard_inputs(inputs, t_loc=T):
    x = np.ascontiguousarray(np.asarray(inputs["x"], np.float32)).reshape(t_loc, B, OBS)
    done = np.ascontiguousarray(np.asarray(inputs["done"], np.float32)).reshape(t_loc, B)
    h0 = np.ascontiguousarray(np.asarray(inputs["gru_state"], np.float32)).reshape(B, H)
    common = {
        k: np.ascontiguousarray(np.asarray(inputs[k], np.float32))
        for k in WEIGHT_KEYS
    }
    in_maps = []
    for c in range(N_CORES):
        sl = slice(c * BL, (c + 1) * BL)
        m = dict(common)
        m["x"] = np.ascontiguousarray(x[:, sl, :])
        m["done"] = np.ascontiguousarray(done[:, sl])
        m["h0"] = np.ascontiguousarray(h0[sl, :])
        in_maps.append(m)
    return in_maps


def assemble_output(per_core_outs, t_loc=T):
    outs = [np.asarray(o, np.float32).reshape(t_loc, BL, A + 1) for o in per_core_outs]
    full = np.stack(outs, axis=1).reshape(t_loc, B, A + 1)
    return np.ascontiguousarray(full.reshape(t_loc * B, A + 1))


def run_on_hw(inputs, t_loc=T, trace=False, **kw):
    from concourse.bass_utils import run_bass_kernel_spmd

    nc = get_built(t_loc)
    in_maps = shard_inputs(inputs, t_loc)
    res = run_bass_kernel_spmd(
        nc, in_maps, core_ids=list(range(N_CORES)), trace=trace, **kw
    )
    out = assemble_output([r["out"] for r in res.results], t_loc)
    return out, res


def kernel(**inputs):
    out, _ = run_on_hw(inputs)
    return out


# revision 14
# speedup vs baseline: 1.1849x; 1.0986x over previous
"""Trainium2 Bass kernel for nn_GruAgent — optimized recurrence chain (v2).

Data-parallel over envs: 8 cores x 64 envs. Per core the GRU runs in
[h-dim on partitions, envs on free] layout. Per-step critical chain is
MM_rz -> sigmoid(r) -> p -> q -> tanh(n) -> t, with:
  - the (1-z)n + z*h state blend and reset-mask folded into MM_rz via
    stacked [U_rz | -U_rz] weights applied to [B; t] where B = z*m*mask'
    and t = n*(z-1)*mask' (so m' = B - t needs no extra hop before it)
  - b_hhn folded into the existing scalar_tensor_tensor that forms
    r*(gh_n + b_hhn)
  - gate activations write both partition halves via the scalar engine
    (the only engine allowed to cross partition bases) so every DVE op
    keeps same-base operands (hardware rule NCC_IBIR297)
  - bulk work (x transpose, gi GEMMs, masks, actor/critic head) is
    chunked and sprinkled between chain steps so the engine FIFOs never
    block the serial recurrence; recurrence/bulk matmuls run in bf16
    (4x the fp32 row rate), PSUM accumulation stays fp32
"""

import os
import sys

import numpy as np

for _p in ("/opt/trn_rl_repo", os.path.expanduser("~/.axon_site/_ro/trn_rl_repo")):
    if os.path.isdir(_p) and _p not in sys.path:
        sys.path.insert(0, _p)
        break

import concourse.bass as bass
import concourse.mybir as mybir
import concourse.tile as tile
from concourse import bacc
from concourse.masks import make_identity

T, B, OBS, H, A, L = 512, 512, 64, 64, 6, 64
N_CORES = 8
BL = B // N_CORES          # 64 envs per core
GS = 8                     # timesteps per group
COLS = GS * BL             # 512 columns per group
H3 = 3 * H

F32 = mybir.dt.float32
BF16 = mybir.dt.bfloat16
AF = mybir.ActivationFunctionType
ALU = mybir.AluOpType

WEIGHT_KEYS = [
    "w_ih", "w_hh", "b_ih", "b_hh",
    "aw1", "ab1", "aw2", "ab2", "aw3", "ab3",
    "cw1", "cb1", "cw2", "cb2", "cw3", "cb3",
]


def build(nc, t_loc=T):
    from contextlib import ExitStack

    assert t_loc % GS == 0
    ng = t_loc // GS

    x_d = nc.dram_tensor("x", [t_loc, BL, OBS], F32, kind="ExternalInput")
    done_d = nc.dram_tensor("done", [t_loc, BL], F32, kind="ExternalInput")
    h0_d = nc.dram_tensor("h0", [BL, H], F32, kind="ExternalInput")
    wih_d = nc.dram_tensor("w_ih", [H3, OBS], F32, kind="ExternalInput")
    whh_d = nc.dram_tensor("w_hh", [H3, H], F32, kind="ExternalInput")
    bih_d = nc.dram_tensor("b_ih", [H3], F32, kind="ExternalInput")
    bhh_d = nc.dram_tensor("b_hh", [H3], F32, kind="ExternalInput")
    aw1_d = nc.dram_tensor("aw1", [L, H + OBS], F32, kind="ExternalInput")
    ab1_d = nc.dram_tensor("ab1", [L], F32, kind="ExternalInput")
    aw2_d = nc.dram_tensor("aw2", [L, L], F32, kind="ExternalInput")
    ab2_d = nc.dram_tensor("ab2", [L], F32, kind="ExternalInput")
    aw3_d = nc.dram_tensor("aw3", [A, L], F32, kind="ExternalInput")
    ab3_d = nc.dram_tensor("ab3", [A], F32, kind="ExternalInput")
    cw1_d = nc.dram_tensor("cw1", [L, H + OBS], F32, kind="ExternalInput")
    cb1_d = nc.dram_tensor("cb1", [L], F32, kind="ExternalInput")
    cw2_d = nc.dram_tensor("cw2", [L, L], F32, kind="ExternalInput")
    cb2_d = nc.dram_tensor("cb2", [L], F32, kind="ExternalInput")
    cw3_d = nc.dram_tensor("cw3", [1, L], F32, kind="ExternalInput")
    cb3_d = nc.dram_tensor("cb3", [1], F32, kind="ExternalInput")
    out_d = nc.dram_tensor("out", [t_loc, BL, A + 1], F32, kind="ExternalOutput")

    with tile.TileContext(nc) as tc, ExitStack() as ctx:
        wp = ctx.enter_context(tc.tile_pool(name="wp", bufs=1))
        ldp = ctx.enter_context(tc.tile_pool(name="ldp", bufs=2))
        xtp = ctx.enter_context(tc.tile_pool(name="xtp", bufs=2))
        pgsp = ctx.enter_context(tc.tile_pool(name="pgsp", bufs=2))
        xnp = ctx.enter_context(tc.tile_pool(name="xnp", bufs=2))
        drp = ctx.enter_context(tc.tile_pool(name="drp", bufs=2))
        mbp = ctx.enter_context(tc.tile_pool(name="mbp", bufs=3))
        hzp = ctx.enter_context(tc.tile_pool(name="hzp", bufs=3))   # hszm group tiles
        small = ctx.enter_context(tc.tile_pool(name="small", bufs=4))
        qp = ctx.enter_context(tc.tile_pool(name="qp", bufs=3))
        tmlp = ctx.enter_context(tc.tile_pool(name="tmlp", bufs=2))
        onp = ctx.enter_context(tc.tile_pool(name="onp", bufs=2))

        przp = ctx.enter_context(tc.tile_pool(name="przp", bufs=2, space="PSUM"))
        pginp = ctx.enter_context(tc.tile_pool(name="pginp", bufs=2, space="PSUM"))
        pghnp = ctx.enter_context(tc.tile_pool(name="pghnp", bufs=2, space="PSUM"))
        pmisc = ctx.enter_context(tc.tile_pool(name="pmisc", bufs=2, space="PSUM"))

        ident = wp.tile([128, 128], F32, tag="ident")
        make_identity(nc, ident[:])

        def load_transposed(dram_ap, rows, cols, tag):
            """dram [rows, cols] -> sbuf tile [cols, rows]."""
            dst = wp.tile([cols, rows], F32, tag=tag)
            r0 = 0
            while r0 < rows:
                rr = min(128, rows - r0)
                tmp = ldp.tile([128, 128], F32, tag="wtmp")
                nc.sync.dma_start(tmp[:rr, :cols], dram_ap[r0:r0 + rr, :])
                pt = pmisc.tile([128, COLS], F32, tag="pm")
                nc.tensor.transpose(pt[:cols, :rr], tmp[:rr, :cols], ident[:rr, :rr])
                nc.scalar.copy(dst[:, r0:r0 + rr], pt[:cols, :rr])
                r0 += rr
            return dst

        def load_col(dram_1d, n, tag, off=0, dst=None, dst_off=0):
            if dst is None:
                dst = wp.tile([max(n + dst_off, 1), 1], F32, tag=tag)
            nc.sync.dma_start(
                dst[dst_off:dst_off + n, :],
                dram_1d[off:off + n].rearrange("p -> p ()"),
            )
            return dst

        # --- weights / constants preprocessing ---
        w_ihT = load_transposed(wih_d[:], H3, OBS, "wihT")    # [64, 192]
        w_hhT = load_transposed(whh_d[:], H3, H, "whhT")      # [64, 192]
        h0T = load_transposed(h0_d[:], BL, H, "h0T")          # [64, 64]

        # [U_rz | -U_rz] stacked along K: [128, 128] (bf16)
        rzUU = wp.tile([128, 128], BF16, tag="rzUU")
        nc.scalar.copy(rzUU[0:64, :], w_hhT[:, 0:128])
        nc.scalar.activation(rzUU[64:128, :], w_hhT[:, 0:128], AF.Identity,
                             scale=-1.0)
        nT_bf = wp.tile([64, 64], BF16, tag="nT_bf")
        nc.scalar.copy(nT_bf[:], w_hhT[:, 128:H3])
        # [U_n | -U_n] so the n-gate matmul reads [B; t] like the rz gate
        nUU = wp.tile([128, 64], BF16, tag="nUU")
        nc.scalar.copy(nUU[0:64, :], w_hhT[:, 128:H3])
        nc.scalar.activation(nUU[64:128, :], w_hhT[:, 128:H3], AF.Identity,
                             scale=-1.0)
        wih_bf = wp.tile([64, H3], BF16, tag="wih_bf")
        nc.scalar.copy(wih_bf[:], w_ihT[:])
        h0T_bf = wp.tile([64, 64], BF16, tag="h0T_bf")
        nc.scalar.copy(h0T_bf[:], h0T[:])

        # head layer1 weights: h-part [64, 128], x-part [64, 128]
        # (cols 0:64 actor, 64:128 critic)
        lhsT1h = wp.tile([64, 128], BF16, tag="lhsT1h")
        lhsT1x = wp.tile([64, 128], BF16, tag="lhsT1x")
        for src, c0 in ((aw1_d, 0), (cw1_d, 64)):
            tmp = ldp.tile([128, 128], F32, tag="wtmp")
            nc.sync.dma_start(tmp[:L, :H + OBS], src[:, :])
            pt = pmisc.tile([128, COLS], F32, tag="pm")
            nc.tensor.transpose(pt[:H, :L], tmp[:L, 0:H], ident[:L, :L])
            nc.tensor.transpose(pt[:OBS, 128:128 + L], tmp[:L, H:H + OBS],
                                ident[:L, :L])
            nc.scalar.copy(lhsT1h[:, c0:c0 + L], pt[:H, :L])
            nc.scalar.copy(lhsT1x[:, c0:c0 + L], pt[:OBS, 128:128 + L])

        lhsT2 = wp.tile([128, 128], BF16, tag="lhsT2")
        nc.vector.memset(lhsT2[:], 0.0)
        for src, o in ((aw2_d, 0), (cw2_d, 64)):
            tmp = ldp.tile([128, 128], F32, tag="wtmp")
            nc.sync.dma_start(tmp[:L, :L], src[:, :])
            pt = pmisc.tile([128, COLS], F32, tag="pm")
            nc.tensor.transpose(pt[:L, :L], tmp[:L, :L], ident[:L, :L])
            nc.scalar.copy(lhsT2[o:o + L, o:o + L], pt[:L, :L])

        lhsT3 = wp.tile([128, A + 1], BF16, tag="lhsT3")
        nc.vector.memset(lhsT3[:], 0.0)
        tmp = ldp.tile([128, 128], F32, tag="wtmp")
        nc.sync.dma_start(tmp[:A, :L], aw3_d[:, :])
        pt = pmisc.tile([128, COLS], F32, tag="pm")
        nc.tensor.transpose(pt[:L, :A], tmp[:A, :L], ident[:A, :A])
        nc.scalar.copy(lhsT3[:L, :A], pt[:L, :A])
        tmp = ldp.tile([128, 128], F32, tag="wtmp")
        nc.sync.dma_start(tmp[:1, :L], cw3_d[:, :])
        pt = pmisc.tile([128, COLS], F32, tag="pm")
        nc.tensor.transpose(pt[:L, :1], tmp[:1, :L], ident[:1, :1])
        nc.scalar.copy(lhsT3[64:64 + L, A:A + 1], pt[:L, :1])

        # biases
        bihc = load_col(bih_d, 128, "bihc")
        bhhc = load_col(bhh_d, 128, "bhhc")
        bias_r = wp.tile([64, 1], F32, tag="bias_r")
        nc.vector.tensor_add(bias_r[:], bihc[0:64, :], bhhc[0:64, :])
        bihz = load_col(bih_d, H, "bihz", off=64)
        bhhz = load_col(bhh_d, H, "bhhz", off=64)
        bias_z = wp.tile([64, 1], F32, tag="bias_z")
        nc.vector.tensor_add(bias_z[:], bihz[:], bhhz[:])
        b_ihn = load_col(bih_d, H, "b_ihn", off=128)          # [64,1]
        b_hhn = load_col(bhh_d, H, "b_hhn", off=128)          # [64,1]

        bias1 = wp.tile([128, 1], F32, tag="bias1")
        load_col(ab1_d, L, "bias1", dst=bias1, dst_off=0)
        load_col(cb1_d, L, "bias1", dst=bias1, dst_off=64)
        bias2 = wp.tile([128, 1], F32, tag="bias2")
        load_col(ab2_d, L, "bias2", dst=bias2, dst_off=0)
        load_col(cb2_d, L, "bias2", dst=bias2, dst_off=64)
        bias3 = wp.tile([A + 1, 1], F32, tag="bias3")
        load_col(ab3_d, A, "bias3", dst=bias3, dst_off=0)
        load_col(cb3_d, 1, "bias3", dst=bias3, dst_off=A)

        ones_row = wp.tile([1, 128], F32, tag="ones_row")
        nc.vector.memset(ones_row[:], 1.0)

        # state tiles (ping-pong): m at partitions 0:64
        mextA = wp.tile([64, BL], BF16, tag="mextA")
        mextB = wp.tile([64, BL], BF16, tag="mextB")

        def bulk_dma(g):
            """Issue the group's DMAs immediately; defer compute into ops."""
            xn = xnp.tile([128, GS // 2, OBS], F32, tag="xn")
            nc.sync.dma_start(
                xn[:],
                x_d[g * GS:(g + 1) * GS].rearrange("(k ph) b f -> (ph b) k f", ph=2),
            )
            dr = drp.tile([1, COLS], F32, tag="dr")
            nc.sync.dma_start(
                dr[:], done_d[g * GS:(g + 1) * GS].rearrange("t b -> () (t b)")
            )
            refs = dict(xn=xn, dr=dr)

            xT = xtp.tile([64, COLS], BF16, tag="xT")
            mb = mbp.tile([128, COLS], BF16, tag="mb")
            prz = przp.tile([128, COLS], F32, tag="prz")
            pgin = pginp.tile([64, COLS], F32, tag="pgin")
            pgin_sb = pgsp.tile([64, COLS], BF16, tag="pgin_sb")
            refs.update(xT=xT, mb=mb, prz=prz, pgin=pgin, pgin_sb=pgin_sb)

            # mask ops run eagerly at the consuming chain's start (they must
            # precede the previous chain's last-step read of mb)
            mask_ops = []
            pmb = pmisc.tile([128, COLS], F32, tag="pm")
            for c in range(2):
                hc = bass.ts(c, COLS // 2)
                mask_ops.append(lambda hc=hc: nc.tensor.matmul(
                    pmb[:, hc], ones_row[:], dr[:, hc], start=True, stop=True,
                    skip_group_check=True))
                mask_ops.append(lambda hc=hc: nc.scalar.activation(
                    mb[:, hc], pmb[:, hc], AF.Identity, scale=-1.0, bias=1.0))
            refs["mask_ops"] = mask_ops

            ops = []
            ptx = pmisc.tile([128, COLS], F32, tag="pm")
            for k in range(GS // 2):
                ops.append(lambda k=k: nc.tensor.transpose(
                    ptx[:OBS, k * 128:(k + 1) * 128], xn[:, k, :], ident[:, :]
                ))
            for c in range(2):
                hc = bass.ts(c, COLS // 2)
                ops.append(lambda hc=hc: nc.vector.tensor_copy(
                    xT[:, hc], ptx[:OBS, hc]))
            # NOTE: start=True marks the whole 2KB PSUM partition-row
            # pending-zero, so only the FIRST chunk may use start=True;
            # later chunks write into already-pending elements (hw zeroes
            # on write) without invalidating earlier chunks.
            for c in range(2):
                hc = bass.ts(c, COLS // 2)
                ops.append(lambda hc=hc, c=c: nc.tensor.matmul(
                    prz[:, hc], wih_bf[:, 0:128], xT[:, hc],
                    start=(c == 0), stop=False, skip_group_check=True,
                ))
                ops.append(lambda hc=hc, c=c: nc.tensor.matmul(
                    pgin[:, hc], wih_bf[:, 128:H3], xT[:, hc],
                    start=(c == 0), stop=(c == 1), skip_group_check=True,
                ))
                ops.append(lambda hc=hc: nc.vector.tensor_copy(
                    pgin_sb[:, hc], pgin[:, hc]))
            return refs, ops

        state = {}

        def chain(g, refs, refs_next, sprinkle):
            prz, mb = refs["prz"], refs["mb"]
            pgin_sb = refs["pgin_sb"]
            hs = hzp.tile([64, COLS], BF16, tag="hs")
            refs["hs"] = hs
            for s in range(GS):
                t = g * GS + s
                cs = bass.ts(s, BL)
                last = t == t_loc - 1
                mext = state["mext"]

                # hidden-state matmuls (rz folded over [B; t], n over m)
                if t > 0:
                    bt = state["bt"]
                    nc.tensor.matmul(
                        prz[:, cs], rzUU[:], bt[:],
                        start=False, stop=(s == GS - 1), skip_group_check=True,
                    )
                else:
                    nc.tensor.matmul(
                        prz[:, cs], rzUU[0:64, :], mext[:],
                        start=False, stop=(s == GS - 1), skip_group_check=True,
                    )
                pghn = pghnp.tile([64, BL], F32, tag="pghn")
                if t > 0:
                    nc.tensor.matmul(
                        pghn[:], nUU[:], state["bt"][:], start=True, stop=True
                    )
                else:
                    nc.tensor.matmul(
                        pghn[:], nT_bf[:], mext[:], start=True, stop=True
                    )

                # gates (ACT is the only engine allowed to cross partition
                # bases, and only with its single tensor input)
                r_lo = small.tile([64, BL], F32, tag="r_lo")
                nc.scalar.activation(
                    r_lo[:], prz[0:64, cs], AF.Sigmoid, bias=bias_r[:]
                )
                z_hi = small.tile([128, BL], BF16, tag="z_hi")
                nc.scalar.activation(
                    z_hi[64:128, :], prz[64:128, cs], AF.Sigmoid, bias=bias_z[:]
                )
                # z/n duplicates at lo come from Pool single-input copies
                # (cross-base is legal for one-input ops; ACT stays lighter)
                z_lo = small.tile([64, BL], BF16, tag="z_lo")
                nc.gpsimd.tensor_copy(z_lo[:], z_hi[64:128, :])
                u_lo = small.tile([64, BL], BF16, tag="u_lo")
                nc.gpsimd.tensor_scalar(
                    u_lo[:], z_lo[:], -1.0, 1.0, ALU.mult, ALU.add,
                )
                # q = r * (U_n m + b_hhn) + gi_n
                p = qp.tile([64, BL], BF16, tag="p")
                nc.vector.scalar_tensor_tensor(
                    p[:], pghn[:], b_hhn[:], r_lo[:], ALU.add, ALU.mult
                )
                q = qp.tile([64, BL], BF16, tag="q")
                nc.vector.tensor_add(q[:], p[:], pgin_sb[:, cs])
                # n at hi (on-path), lo duplicate via Pool copy
                n_hi = small.tile([128, BL], BF16, tag="n_hi")
                nc.scalar.activation(
                    n_hi[64:128, :], q[:], AF.Tanh, bias=b_ihn[:],
                )
                n_lo = small.tile([64, BL], BF16, tag="n_lo")
                nc.gpsimd.tensor_copy(n_lo[:], n_hi[64:128, :])
                # zm = z*m (head + B)
                zm = small.tile([64, BL], BF16, tag="zm")
                nc.vector.tensor_mul(zm[:], z_lo[:], mext[:])

                if not last:
                    if s == GS - 1:
                        mbn = refs_next["mb"]
                        csn = bass.ts(0, BL)
                    else:
                        mbn = mb
                        csn = bass.ts(s + 1, BL)
                    bt = small.tile([128, BL], BF16, tag="bt")
                    # B = zm*mask' at lo -> bt[0:]
                    nc.vector.tensor_mul(bt[0:64, :], zm[:], mbn[0:64, csn])
                    # A = (z-1)*mask' at hi; t = n*A -> bt[64:] (on-path)
                    a_hi = small.tile([128, BL], BF16, tag="a_hi")
                    nc.vector.scalar_tensor_tensor(
                        a_hi[64:128, :], z_hi[64:128, :], -1.0, mbn[64:128, csn],
                        ALU.add, ALU.mult,
                    )
                    nc.vector.tensor_mul(bt[64:128, :], n_hi[64:128, :],
                                         a_hi[64:128, :])
                    # off-path: t at lo, m' = B - t_lo (feeds zm next step)
                    a_lo = small.tile([64, BL], BF16, tag="a_lo")
                    nc.vector.scalar_tensor_tensor(
                        a_lo[:], z_lo[:], -1.0, mbn[0:64, csn],
                        ALU.add, ALU.mult,
                    )
                    t_lo = small.tile([64, BL], BF16, tag="t_lo")
                    nc.vector.tensor_mul(t_lo[:], n_lo[:], a_lo[:])
                    mext2 = mextA if ((t + 1) % 2 == 0) else mextB
                    nc.vector.tensor_sub(mext2[:], bt[0:64, :], t_lo[:])
                    state["bt"] = bt
                    state["mext"] = mext2
                # hs for the head (fully off the recurrence path)
                t2 = small.tile([64, BL], BF16, tag="t2")
                nc.gpsimd.tensor_mul(t2[:], n_lo[:], u_lo[:])
                nc.gpsimd.tensor_add(hs[:, cs], t2[:], zm[:])
                # emit a slice of the background queue after each step
                rem = GS - s
                k = (len(sprinkle) + rem - 1) // rem
                for _ in range(min(k, len(sprinkle))):
                    sprinkle.pop(0)()
            while sprinkle:
                sprinkle.pop(0)()

        def head_ops(g, refs):
            hszm, xT = refs["hs"], refs["xT"]
            p1 = pmisc.tile([128, COLS], F32, tag="pm")
            t1 = tmlp.tile([128, COLS], BF16, tag="t1")
            p2 = pmisc.tile([128, COLS], F32, tag="pm")
            t2 = tmlp.tile([128, COLS], BF16, tag="t2")
            p3 = pmisc.tile([128, COLS], F32, tag="pm")
            o7 = tmlp.tile([A + 1, COLS], F32, tag="o7")
            ops = []
            for c in range(2):
                hc = bass.ts(c, COLS // 2)
                ops.append(lambda hc=hc: nc.tensor.matmul(
                    p1[:, hc], lhsT1h[:], hszm[:, hc], start=True, stop=False,
                    skip_group_check=True))
                ops.append(lambda hc=hc: nc.tensor.matmul(
                    p1[:, hc], lhsT1x[:], xT[:, hc], start=False, stop=True,
                    skip_group_check=True))
                ops.append(lambda hc=hc: nc.scalar.activation(
                    t1[:, hc], p1[:, hc], AF.Tanh, bias=bias1[:]))
                ops.append(lambda hc=hc: nc.tensor.matmul(
                    p2[:, hc], lhsT2[:], t1[:, hc], start=True, stop=True,
                    skip_group_check=True))
                ops.append(lambda hc=hc: nc.scalar.activation(
                    t2[:, hc], p2[:, hc], AF.Tanh, bias=bias2[:]))
                ops.append(lambda hc=hc: nc.tensor.matmul(
                    p3[:A + 1, hc], lhsT3[:], t2[:, hc], start=True, stop=True,
                    skip_group_check=True))
                ops.append(lambda hc=hc: nc.scalar.activation(
                    o7[:, hc], p3[:A + 1, hc], AF.Identity, bias=bias3[:]))
            po = pmisc.tile([128, GS // 2, A + 1], F32, tag="pm")
            for k in range(GS // 2):
                ops.append(lambda k=k: nc.tensor.transpose(
                    po[:, k, :], o7[:, k * 128:(k + 1) * 128],
                    ident[:A + 1, :A + 1]))
            on = onp.tile([128, GS // 2, A + 1], F32, tag="on")
            for c in range(2):
                ops.append(lambda c=c: nc.vector.tensor_copy(
                    on[:, c * 2:(c + 1) * 2, :], po[:, c * 2:(c + 1) * 2, :]))
            ops.append(lambda: nc.sync.dma_start(
                out_d[g * GS:(g + 1) * GS].rearrange(
                    "(k ph) b j -> (ph b) k j", ph=2),
                on[:],
            ))
            return ops

        all_refs = {}
        all_refs[0], ops0 = bulk_dma(0)
        for op in all_refs[0]["mask_ops"] + ops0:
            op()
        # m_0 = mask_0 * h0  into mextA
        nc.vector.tensor_mul(mextA[:], h0T_bf[:], all_refs[0]["mb"][0:64, 0:BL])
        state["mext"] = mextA
        prev_head = []
        for g in range(ng):
            if g + 1 < ng:
                all_refs[g + 1], bops = bulk_dma(g + 1)
                # next group's mask must exist before this chain's last step
                for op in all_refs[g + 1]["mask_ops"]:
                    op()
            else:
                bops = []
            # head ops first: they reuse pmisc buffers that the later bulk
            # ops of the following group will overwrite (emission order is
            # program order, so readers must be emitted before new writers)
            chain(g, all_refs[g], all_refs.get(g + 1), prev_head + bops)
            prev_head = head_ops(g, all_refs[g])
            all_refs.pop(g - 1, None)
        for op in prev_head:
            op()

    return nc


_BUILT = {}


def get_built(t_loc=T):
    if t_loc not in _BUILT:
        nc = bacc.Bacc(None, target_bir_lowering=False)
        build(nc, t_loc)
        nc.compile()
        _BUILT[t_loc] = nc
    return _BUILT[t_loc]


def shard_inputs(inputs, t_loc=T):
    x = np.ascontiguousarray(np.asarray(inputs["x"], np.float32)).reshape(t_loc, B, OBS)
    done = np.ascontiguousarray(np.asarray(inputs["done"], np.float32)).reshape(t_loc, B)
    h0 = np.ascontiguousarray(np.asarray(inputs["gru_state"], np.float32)).reshape(B, H)
    common = {
        k: np.ascontiguousarray(np.asarray(inputs[k], np.float32))
        for k in WEIGHT_KEYS
    }
    in_maps = []
    for c in range(N_CORES):
        sl = slice(c * BL, (c + 1) * BL)
        m = dict(common)
        m["x"] = np.ascontiguousarray(x[:, sl, :])
        m["done"] = np.ascontiguousarray(done[:, sl])
        m["h0"] = np.ascontiguousarray(h0[sl, :])
        in_maps.append(m)
    return in_maps


def assemble_output(per_core_outs, t_loc=T):
    outs = [np.asarray(o, np.float32).reshape(t_loc, BL, A + 1) for o in per_core_outs]
    full = np.stack(outs, axis=1).reshape(t_loc, B, A + 1)
    return np.ascontiguousarray(full.reshape(t_loc * B, A + 1))


def run_on_hw(inputs, t_loc=T, trace=False, **kw):
    from concourse.bass_utils import run_bass_kernel_spmd

    nc = get_built(t_loc)
    in_maps = shard_inputs(inputs, t_loc)
    res = run_bass_kernel_spmd(
        nc, in_maps, core_ids=list(range(N_CORES)), trace=trace, **kw
    )
    out = assemble_output([r["out"] for r in res.results], t_loc)
    return out, res


def kernel(**inputs):
    out, _ = run_on_hw(inputs)
    return out
